# revision 21
# baseline (speedup 1.0000x reference)
"""Trainium2 Bass kernel for CellGraphSignatureGNN (GCN message passing).

Math: the network is affine per layer: x_{l+1} = A @ x_l @ W_l + 1 b_l^T,
with A = D^-1/2 (Adj + 2I) D^-1/2 (weighted adjacency + improved self loops),
followed by a per-graph mean pool P (and bs == 0 in this problem).  Since A
acts on nodes and W on features:

    out = P A^3 X (W0 W1 W2) / counts

We evaluate left-to-right: u1^T = A^T P^T, u2^T = A^T u1^T, u3^T = A^T u2^T
(64-wide node vectors), then one dense matmul (u3 X) sharded over nodes, a
tiny AllReduce, and the 128x128 weight chain on-device.

Distribution: nodes are re-labeled and bin-packed into 128-node blocks (100
blocks/core x 8 cores) balanced by scatter-degree so every core runs an
IDENTICAL (SPMD) program; per-core data (gather indices, one-hots, norms)
differ.  Edge (r, c) contributes norm_e * u[c] into u_new[r]:
  - each core's shard quarter is AllGathered separately (4 quarter-AGs per
    pass) into window buffers uq[w]; passes 1-2 process window-major so
    window-w work only waits for AG w (pipelines collectives with compute),
  - gather u[c] row-pairs (256B) from uq[w] via dma_gather issued as
    prepare_only descriptor preps + trigger_dma (descriptor generation on
    GpSimd decouples from SDMA drain), round-robined over 4 SWDGE queues,
  - slots are parity-packed: edges whose source row is even sit in
    partitions 0-63, odd in 64-127, so the even/odd pair-half selection is
    done by the scatter matmuls themselves (no DVE combine),
  - scatter = one-hot matmul accumulated in PSUM per 128-dest block; for
    OHL groups the one-hot comes scaled (norm folded) from HBM, for the
    rest it is built 0/1 on DVE (is_eq) and the norm is applied to the
    gathered messages with a single DVE multiply (balances DVE vs DMA),
  - (block, window) segments have VARIABLE chunk counts: a segment whose
    per-parity edge count exceeds 64*chunks on ANY core gets an extra
    chunk (max across cores keeps the program core-independent), so there
    is no spill/virtual-block machinery at all.
Self-loops are applied as an in-place elementwise multiply on the
SBUF-resident accumulator.
"""

import numpy as np
import ml_dtypes

BF16 = ml_dtypes.bfloat16

G = 64        # graphs
F = 128       # feature width
LAYERS = 3
PAD_SENT = 30000.0  # destoff/bg sentinel for padded slots (one-hot -> 0)


# --------------------------------------------------------------------------
# configuration
# --------------------------------------------------------------------------
class Cfg:
    def __init__(self, n_nodes, n_edges, n_cores=8, nblk=100, group_sizes=None,
                 base_chunks=2, max_chunks=4, n_win=4, ohl_groups=(0, 2, 4)):
        self.n_nodes = n_nodes
        self.n_edges = n_edges
        self.n_cores = n_cores
        self.nblk = nblk                      # blocks per core
        self.group_sizes = group_sizes or [13, 12] * 4
        assert sum(self.group_sizes) == nblk
        self.base_chunks = base_chunks        # min chunks per (block, window)
        self.max_chunks = max_chunks
        self.n_win = n_win
        self.ohl_groups = set(ohl_groups)     # groups w/ HBM-loaded scaled oh
        self.core_rows = nblk * 128
        self.pn = n_cores * self.core_rows
        assert self.core_rows % n_win == 0
        self.qrows = self.core_rows // n_win  # rows per (core, quarter)
        self.win = n_cores * self.qrows       # rows per assembled window
        assert self.win // 2 <= 32767         # pair index must fit int16
        assert nblk % n_win == 0
        self.blocks_per_q = nblk // n_win
        assert len(self.group_sizes) == 2 * n_win
        for q in range(n_win):
            assert (self.group_sizes[2 * q] + self.group_sizes[2 * q + 1]
                    == self.blocks_per_q)


FULL_CFG = Cfg(100000, 640000)


class Layout:
    """Slot layout with per-(block, window) chunk counts, identical on all
    cores.  Order: for g in groups: for w in windows: for j in group: chunks.
    """

    def __init__(self, cfg, c_bw):
        self.c_bw = c_bw                      # [nblk, n_win] chunks
        blk_group, blk_ing = [], []
        for g, gs in enumerate(cfg.group_sizes):
            for j in range(gs):
                blk_group.append(g)
                blk_ing.append(j)
        self.blk_group = np.array(blk_group)
        self.blk_ing = np.array(blk_ing)
        self.gbase = [0]
        for gs in cfg.group_sizes:
            self.gbase.append(self.gbase[-1] + gs)
        self.gw_coff = {}                     # (g, w) -> first chunk
        self.gw_nc = {}                       # (g, w) -> n chunks
        self.blk_c0 = {}                      # (b, w) -> first chunk (abs)
        off = 0
        for g, gs in enumerate(cfg.group_sizes):
            for w in range(cfg.n_win):
                self.gw_coff[(g, w)] = off
                n = 0
                for j in range(gs):
                    b = self.gbase[g] + j
                    self.blk_c0[(b, w)] = off + n
                    n += int(c_bw[b, w])
                self.gw_nc[(g, w)] = n
                off += n
        self.n_chunks = off
        self.n_slots = off * 128

    def key(self):
        return self.c_bw.tobytes()


# --------------------------------------------------------------------------
# host-side graph preprocessing (indices, norms, schedules)
# --------------------------------------------------------------------------
def host_prep(cfg, x, edge_index, edge_attr, batch, Ws, bs):
    N, E = cfg.n_nodes, cfg.n_edges
    row = np.asarray(edge_index[0], dtype=np.int64)
    col = np.asarray(edge_index[1], dtype=np.int64)
    w = np.asarray(edge_attr, dtype=np.float32).reshape(-1)
    batch = np.asarray(batch, dtype=np.int64)

    deg = np.zeros(N, dtype=np.float64)
    np.add.at(deg, col, w.astype(np.float64))
    deg += 2.0
    dinv = (1.0 / np.sqrt(deg)).astype(np.float32)
    norm = dinv[row] * w * dinv[col]
    selfnorm = 2.0 * dinv * dinv
    cnt = np.bincount(batch, minlength=G).astype(np.float32)

    # ---- bin-pack nodes into blocks by scatter degree (edges with row == n)
    sdeg = np.bincount(row, minlength=N)
    nbins = cfg.n_cores * cfg.nblk
    order = np.argsort(-sdeg, kind="stable")
    binsum = np.zeros(nbins, dtype=np.int64)
    binfill = np.zeros(nbins, dtype=np.int32)
    import heapq
    heap = [(0, b) for b in range(nbins)]
    heapq.heapify(heap)
    node_bin = np.empty(N, dtype=np.int32)
    node_pos = np.empty(N, dtype=np.int32)
    for n in order:
        while True:
            s, b = heapq.heappop(heap)
            if binfill[b] < 128:
                break
        node_bin[n] = b
        node_pos[n] = binfill[b]
        binfill[b] += 1
        binsum[b] += sdeg[n]
        if binfill[b] < 128:
            heapq.heappush(heap, (int(binsum[b]), b))
    # snake-assign bins to cores by load
    border = np.argsort(-binsum, kind="stable")
    bin_core = np.empty(nbins, dtype=np.int32)
    bin_blk = np.empty(nbins, dtype=np.int32)
    percore = [[] for _ in range(cfg.n_cores)]
    for i, b in enumerate(border):
        r = i // cfg.n_cores
        k = i % cfg.n_cores
        c = k if (r % 2 == 0) else cfg.n_cores - 1 - k
        bin_core[b] = c
        bin_blk[b] = len(percore[c])
        percore[c].append(b)
    assert all(len(p) == cfg.nblk for p in percore)

    # local row within core; window/widx mapping depends on the AG scheme:
    #  - quarter AGs: window = node's local quarter; widx = row in uq[w]
    #    (= core * qrows + row-within-quarter)
    #  - full AG: window = core pair; widx = row in ufull[w*win:(w+1)*win]
    local_row = bin_blk[node_bin].astype(np.int64) * 128 + node_pos
    node_core = bin_core[node_bin].astype(np.int64)
    if QUARTER_AG:
        node_q = local_row // cfg.qrows
        node_widx = node_core * cfg.qrows + (local_row % cfg.qrows)
    else:
        node_q = node_core // 2
        node_widx = (node_core % 2) * cfg.core_rows + local_row

    # ---- per-core edge schedules
    e_core = bin_core[node_bin[row]]
    e_blk = bin_blk[node_bin[row]]
    e_doff = node_pos[row]                 # dest offset within block
    e_win = node_q[col].astype(np.int32)   # source window (quarter)
    e_gidx = node_widx[col].astype(np.int32)  # gather idx within window

    n_cores = cfg.n_cores

    # per-core, per-(block, window, parity) counts -> chunk requirements
    percore_data = []
    need = np.full((cfg.nblk, cfg.n_win), cfg.base_chunks, dtype=np.int32)
    for c in range(n_cores):
        em = e_core == c
        eb = e_blk[em]
        ew = e_win[em]
        eg = e_gidx[em]
        ed = e_doff[em]
        en = norm[em]
        ebg = batch[col[em]].astype(np.float32)
        key = (eb * cfg.n_win + ew) * 2 + (eg & 1)
        o = np.argsort(key, kind="stable")
        eb, ew, eg, ed, en, ebg = eb[o], ew[o], eg[o], ed[o], en[o], ebg[o]
        kk = key[o]
        bounds = np.searchsorted(kk, np.arange(cfg.nblk * cfg.n_win * 2 + 1))
        cnts = np.diff(bounds).reshape(cfg.nblk, cfg.n_win, 2)
        req = (np.ceil(cnts.max(axis=2) / 64.0)).astype(np.int32)
        need = np.maximum(need, req)
        percore_data.append((eb, ew, eg, ed, en, ebg, bounds))
    assert need.max() <= cfg.max_chunks, f"segment overflow: {need.max()}"
    lay = Layout(cfg, need)
    S = lay.n_slots
    SC = lay.n_chunks

    gidx = np.zeros((n_cores, S), dtype=np.int32)   # pair index (widx // 2)
    doff = np.full((n_cores, S), PAD_SENT, dtype=np.float32)
    nrm = np.zeros((n_cores, S), dtype=np.float32)
    bg = np.full((n_cores, S), PAD_SENT, dtype=np.float32)

    for c in range(n_cores):
        eb, ew, eg, ed, en, ebg, bounds = percore_data[c]
        for b in range(cfg.nblk):
            for wi in range(cfg.n_win):
                base = lay.blk_c0[(b, wi)] * 128
                for par in range(2):
                    i0 = bounds[(b * cfg.n_win + wi) * 2 + par]
                    i1 = bounds[(b * cfg.n_win + wi) * 2 + par + 1]
                    n = i1 - i0
                    if n == 0:
                        continue
                    k = np.arange(n)
                    sl = base + (k // 64) * 128 + 64 * par + (k % 64)
                    gidx[c, sl] = eg[i0:i1] >> 1
                    doff[c, sl] = ed[i0:i1]
                    nrm[c, sl] = en[i0:i1]
                    bg[c, sl] = ebg[i0:i1]

    # ---- pack aux arrays
    ncol16 = S // 16
    gidx16 = np.zeros((n_cores, 128, ncol16), dtype=np.int16)
    s_idx = np.arange(S)
    for c in range(n_cores):
        layi = np.zeros((16, ncol16), dtype=np.int16)
        layi[s_idx % 16, s_idx // 16] = gidx[c].astype(np.int16)
        gidx16[c] = np.tile(layi, (8, 1))

    def slotmajor(a, dt):
        out = np.zeros((n_cores, 128, SC), dtype=dt)
        for c in range(n_cores):
            out[c][s_idx % 128, s_idx // 128] = a[c]
        return out

    nrm_sm = slotmajor(nrm, BF16)
    doff_sm = slotmajor(doff, BF16)

    # which slots belong to OHL (scaled one-hot from HBM) groups
    ohl_slot = np.zeros(S, dtype=bool)
    for g in cfg.ohl_groups:
        for wi in range(cfg.n_win):
            off = lay.gw_coff[(g, wi)] * 128
            ohl_slot[off:off + lay.gw_nc[(g, wi)] * 128] = True

    # host-prebuilt one-hots: scaled (norm folded) for OHL slots; and pass-0
    # messages: scaled for non-OHL slots (their device one-hot is 0/1),
    # unscaled for OHL slots (their one-hot carries the norm).
    p_i = s_idx % 128
    c_i = s_idx // 128
    oh_hbm = np.zeros((n_cores, 128, SC, 128), dtype=BF16)
    msg0_hbm = np.zeros((n_cores, 128, SC, G), dtype=BF16)
    for c in range(n_cores):
        dv = doff[c].astype(np.int64)
        m = (doff[c] != PAD_SENT) & ohl_slot
        oh_hbm[c][p_i[m], c_i[m], dv[m]] = nrm[c][m].astype(BF16)
        bv = bg[c].astype(np.int64)
        mb = bg[c] != PAD_SENT
        val = np.where(ohl_slot, 1.0, nrm[c]).astype(np.float32)
        msg0_hbm[c][p_i[mb], c_i[mb], bv[mb]] = val[mb].astype(BF16)

    # ---- per-core node-level aux
    selfw = np.zeros((n_cores, 128, cfg.nblk), dtype=np.float32)
    batchloc = np.full((n_cores, 128, cfg.nblk), PAD_SENT, dtype=BF16)
    Xp = np.zeros((n_cores, cfg.core_rows, F), dtype=np.float32)
    nodes = np.arange(N)
    pc = bin_core[node_bin]
    pb = bin_blk[node_bin]
    pp = node_pos
    for c in range(n_cores):
        m = pc == c
        selfw[c][pp[m], pb[m]] = selfnorm[nodes[m]]
        batchloc[c][pp[m], pb[m]] = batch[nodes[m]].astype(np.float32)
        Xp[c][pb[m] * 128 + pp[m]] = np.asarray(x, dtype=np.float32)[nodes[m]]

    inv_cnt = (1.0 / np.maximum(cnt, 1.0)).astype(np.float32).reshape(G, 1)
    Ws = np.asarray(Ws, dtype=np.float32)
    bs = np.asarray(bs, dtype=np.float32)

    aux = dict(
        layout=lay,
        gidx16=gidx16, nrm_sm=nrm_sm, doff_sm=doff_sm,
        Xpb=Xp.astype(BF16),
        oh_hbm=oh_hbm, msg0_hbm=msg0_hbm,
        selfw=selfw, batchloc=batchloc, Xp=Xp,
        inv_cnt=inv_cnt,
        W0T=np.ascontiguousarray(Ws[0].T), W1T=np.ascontiguousarray(Ws[1].T),
        W2=np.ascontiguousarray(Ws[2]), bs=bs,
    )
    return aux


def _midbcast(ap, count):
    """Insert a step-0 middle axis: [P, X] -> [P, (0,count), X]."""
    import concourse.bass as bass
    assert len(ap.ap) == 2
    return bass.AP(ap.tensor, ap.offset, [ap.ap[0], [0, count], ap.ap[1]])


USE_PREP = False      # prepare_only + trigger_dma gathers
QUARTER_AG = True     # 4 quarter AllGathers per pass (else one full AG)
PARITY_MM = False     # pair-half selection via 64-row matmuls (else DVE)


def build_program(cfg, lay):
    import contextlib
    import concourse.bacc as bacc
    import concourse.mybir as mybir
    import concourse.tile as tile

    f32 = mybir.dt.float32
    bf16 = mybir.dt.bfloat16
    i16 = mybir.dt.int16
    AL = mybir.AluOpType

    S = lay.n_slots
    SC = lay.n_chunks
    NBLK = cfg.nblk
    NW = cfg.n_win
    BPQ = cfg.blocks_per_q
    QR = cfg.qrows
    gbase = lay.gbase

    nc = bacc.Bacc("TRN2", debug=False, num_devices=cfg.n_cores,
                   num_swdge_queues=4)
    P = nc.declare_dram_parameter

    gidx16 = P("gidx16", [128, S // 16], i16, isOutput=False)
    nrm_sm = P("nrm_sm", [128, SC], bf16, isOutput=False)
    doff_sm = P("doff_sm", [128, SC], bf16, isOutput=False)
    oh_hbm = P("oh_hbm", [128, SC, 128], bf16, isOutput=False)
    msg0_hbm = P("msg0_hbm", [128, SC, G], bf16, isOutput=False)
    selfw = P("selfw", [128, NBLK], f32, isOutput=False)
    batchloc = P("batchloc", [128, NBLK], bf16, isOutput=False)
    Xp = P("Xpb", [cfg.core_rows, F], bf16, isOutput=False)
    inv_cnt = P("inv_cnt", [G, 1], f32, isOutput=False)
    W0T = P("W0T", [F, F], f32, isOutput=False)
    W1T = P("W1T", [F, F], f32, isOutput=False)
    W2 = P("W2", [F, F], f32, isOutput=False)
    out_ext = P("out", [G, F], f32, isOutput=True)

    # node-sharded output (AG inputs) and window-replicated u (AG outputs)
    if QUARTER_AG:
        shardq = [nc.dram_tensor(f"shardq{q}", [QR, G], bf16)
                  for q in range(NW)]
        uq = [[nc.dram_tensor(f"uq{p}_{w}", [cfg.win, G], bf16)
               for w in range(NW)] for p in range(2)]
    else:
        shard = nc.dram_tensor("shard", [cfg.core_rows, G], bf16)
        ufull = [nc.dram_tensor(f"ufull{p}", [cfg.pn, G], bf16)
                 for p in range(2)]
    arin = nc.dram_tensor("arin", [G, F], f32)
    arout = nc.dram_tensor("arout", [G, F], f32)

    iota64_c = nc.inline_tensor(
        np.tile(np.arange(G, dtype=np.float32).astype(BF16), (128, 1)), "iota64")
    iota128_c = nc.inline_tensor(
        np.tile(np.arange(128, dtype=np.float32).astype(BF16), (128, 1)),
        "iota128")
    ident_c = nc.inline_tensor(np.eye(128, dtype=np.float32), "ident")

    qn = [0]

    def next_q():
        qn[0] = (qn[0] + 1) % 4
        return qn[0]

    # SWDGE DMA-completion sems: one per Tile DMASW lane (8), assigned in
    # prep-emission order to match tile_sem_assignment's lane rotation.
    swsems = [nc.alloc_semaphore(f"sws{i}") for i in range(8)]
    sem_n = [0]

    def next_sem():
        s = swsems[sem_n[0] % 8]
        sem_n[0] += 1
        return s

    max_gs = max(cfg.group_sizes)
    max_c = max(lay.gw_nc.values())

    with tile.TileContext(nc) as tc:
        with contextlib.ExitStack() as ctx:
            perm_pool = ctx.enter_context(tc.tile_pool(name="perm", bufs=1))
            acc = perm_pool.tile([128, NBLK, G], bf16, tag="acc")
            selfw_sb = perm_pool.tile([128, NBLK], f32, tag="selfw")
            blc_sb = perm_pool.tile([128, NBLK], bf16, tag="blc")
            io64 = perm_pool.tile([128, G], bf16, tag="io64")
            io128 = perm_pool.tile([128, 128], bf16, tag="io128")
            ident_sb = perm_pool.tile([128, 128], f32, tag="ident")
            w_sb = perm_pool.tile([128, 3 * F], f32, tag="wsb")

            nc.sync.dma_start(out=selfw_sb[:], in_=selfw[:])
            nc.sync.dma_start(out=blc_sb[:], in_=batchloc[:])
            nc.sync.dma_start(out=io64[:], in_=iota64_c[:])
            nc.sync.dma_start(out=io128[:], in_=iota128_c[:])
            nc.sync.dma_start(out=ident_sb[:], in_=ident_c[:])
            nc.sync.dma_start(out=w_sb[:, 0:F], in_=W0T[:])
            nc.sync.dma_start(out=w_sb[:, F:2 * F], in_=W1T[:])
            nc.sync.dma_start(out=w_sb[:, 2 * F:3 * F], in_=W2[:])

            raw_pool = ctx.enter_context(tc.tile_pool(name="raw", bufs=5))
            msg_pool = ctx.enter_context(tc.tile_pool(name="msg", bufs=4))
            oh_pool = ctx.enter_context(tc.tile_pool(name="oh", bufs=5))
            ps_pool = ctx.enter_context(tc.tile_pool(name="ps", bufs=2, space="PSUM"))
            ep_pool = ctx.enter_context(tc.tile_pool(name="ep", bufs=2, space="PSUM"))
            fin_pool = ctx.enter_context(tc.tile_pool(name="fin", bufs=2))
            xp_pool = ctx.enter_context(tc.tile_pool(name="xp", bufs=2))

            # static per-slot aux data, resident in SBUF for all passes
            gidx_sb = perm_pool.tile([128, S // 16], i16, tag="gidx")
            nrm_sb = perm_pool.tile([128, SC], bf16, tag="nrmsb")
            doff_sb = perm_pool.tile([128, SC], bf16, tag="doffsb")
            nc.sync.dma_start(out=gidx_sb[:], in_=gidx16[:])
            nc.sync.dma_start(out=nrm_sb[:], in_=nrm_sm[:])
            nc.sync.dma_start(out=doff_sb[:], in_=doff_sm[:])

            # epilogue weight chain early (overlaps pass 0):
            # W12 = W1 @ W2 ; W012 = W0 @ W12
            wps = ep_pool.tile([128, F], f32, tag="ep")
            w12 = fin_pool.tile([128, F], f32, tag="w12")
            nc.tensor.matmul(wps[:], lhsT=w_sb[:, F:2 * F],
                             rhs=w_sb[:, 2 * F:3 * F], start=True, stop=True)
            nc.vector.tensor_copy(out=w12[:], in_=wps[:])
            wps2 = ep_pool.tile([128, F], f32, tag="ep")
            w012 = perm_pool.tile([128, F], f32, tag="w012")
            nc.tensor.matmul(wps2[:], lhsT=w_sb[:, 0:F], rhs=w12[:],
                             start=True, stop=True)
            nc.vector.tensor_copy(out=w012[:], in_=wps2[:])

            def gather_raw(pk, g, w):
                """prepare_only gather of the (g, w) slot region's pairs."""
                C = lay.gw_nc[(g, w)]
                soff = lay.gw_coff[(g, w)] * 128
                if QUARTER_AG:
                    usrc = uq[(pk + 1) % 2][w][:]
                else:
                    usrc = ufull[(pk + 1) % 2][
                        w * 2 * cfg.core_rows:(w + 1) * 2 * cfg.core_rows, :]
                src = usrc.rearrange("(p two) f -> p (two f)", two=2)
                rawt = raw_pool.tile([128, max_c, 2 * G], bf16, tag="raw")
                CSUB = 8   # <=1024 descriptors per call (SWDGE ring limit)
                for sub in range(0, C, CSUB):
                    cs = min(CSUB, C - sub)
                    q = next_q()
                    if USE_PREP:
                        nc.gpsimd.dma_gather(
                            rawt[:, sub:sub + cs, :], src,
                            gidx_sb[:, soff // 16 + sub * 8:
                                    soff // 16 + (sub + cs) * 8],
                            cs * 128, cs * 128, 2 * G,
                            single_packet=False, queue_num=q,
                            prepare_only=True, sem=next_sem())
                        nc.gpsimd.trigger_dma(count=None, queue_num=q)
                    else:
                        nc.gpsimd.dma_gather(
                            rawt[:, sub:sub + cs, :], src,
                            gidx_sb[:, soff // 16 + sub * 8:
                                    soff // 16 + (sub + cs) * 8],
                            cs * 128, cs * 128, 2 * G,
                            single_packet=False, queue_num=q)
                return rawt

            def build_oh(g, w):
                C = lay.gw_nc[(g, w)]
                coff = lay.gw_coff[(g, w)]
                oht = oh_pool.tile([128, max_c, 128], bf16, tag="oh")
                if g in cfg.ohl_groups:
                    nc.scalar.dma_start(out=oht[:, 0:C, :],
                                        in_=oh_hbm[:, coff:coff + C, :])
                else:
                    nc.vector.tensor_tensor(
                        out=oht[:, 0:C, :],
                        in0=doff_sb[:, coff:coff + C].to_broadcast(
                            [128, C, 128]),
                        in1=_midbcast(io128[:], C),
                        op=AL.is_equal)
                return oht

            def build_msg(pk, g, w, rawt):
                """rhs tensor for the scatter matmuls + whether parity-split."""
                C = lay.gw_nc[(g, w)]
                coff = lay.gw_coff[(g, w)]
                if pk == 0:
                    msgt = msg_pool.tile([128, max_c, G], bf16, tag="msg0")
                    nc.sync.dma_start(out=msgt[:, 0:C, :],
                                      in_=msg0_hbm[:, coff:coff + C, :])
                    return msgt, False
                if PARITY_MM:
                    if g in cfg.ohl_groups:
                        return rawt, True
                    msgt = msg_pool.tile([128, max_c, 2 * G], bf16, tag="msg")
                    nc.vector.tensor_tensor(
                        out=msgt[:, 0:C, :], in0=rawt[:, 0:C, :],
                        in1=nrm_sb[:, coff:coff + C].to_broadcast(
                            [128, C, 2 * G]),
                        op=AL.mult)
                    return msgt, True
                # DVE pair-half selection: partitions 0-63 take the even half,
                # 64-127 the odd half; single full-contraction matmuls follow.
                msgt = msg_pool.tile([128, max_c, G], bf16, tag="msgs")
                for par in range(2):
                    p0, p1 = 64 * par, 64 * (par + 1)
                    src = rawt[p0:p1, 0:C, par * G:(par + 1) * G]
                    if g in cfg.ohl_groups:
                        nc.vector.tensor_copy(out=msgt[p0:p1, 0:C, :], in_=src)
                    else:
                        nc.vector.tensor_tensor(
                            out=msgt[p0:p1, 0:C, :], in0=src,
                            in1=nrm_sb[p0:p1, coff:coff + C].to_broadcast(
                                [64, C, G]),
                            op=AL.mult)
                return msgt, False

            def group_window(pk, g, w):
                """Process (group g, window w): gather, oh, msgs, matmuls."""
                gs = cfg.group_sizes[g]
                seg = slice(gbase[g], gbase[g] + gs)
                coff = lay.gw_coff[(g, w)]
                rawt = gather_raw(pk, g, w) if pk > 0 else None
                oht = build_oh(g, w)
                msgt, split = build_msg(pk, g, w, rawt)
                ps = ps_pool.tile([128, max_gs * G], f32, tag="ps")
                for j in range(gs):
                    b = gbase[g] + j
                    c0 = lay.blk_c0[(b, w)] - coff
                    nchunk = int(lay.c_bw[b, w])
                    n = nchunk * (2 if split else 1)
                    k = 0
                    for ci in range(c0, c0 + nchunk):
                        if split:
                            for par in range(2):
                                p0, p1 = 64 * par, 64 * (par + 1)
                                nc.tensor.matmul(
                                    ps[:, j * G:(j + 1) * G],
                                    lhsT=oht[p0:p1, ci, :],
                                    rhs=msgt[p0:p1, ci, par * G:(par + 1) * G],
                                    start=(k == 0), stop=(k == n - 1))
                                k += 1
                        else:
                            nc.tensor.matmul(
                                ps[:, j * G:(j + 1) * G],
                                lhsT=oht[:, ci, :], rhs=msgt[:, ci, :],
                                start=(k == 0), stop=(k == n - 1))
                            k += 1
                nc.vector.tensor_tensor(
                    out=acc[:, seg, :], in0=acc[:, seg, :],
                    in1=ps[:, :gs * G].rearrange("p (b f) -> p b f", f=G),
                    op=AL.add)

            def self_init(pk, g):
                seg = slice(gbase[g], gbase[g] + cfg.group_sizes[g])
                gs = cfg.group_sizes[g]
                if pk == 0:
                    nc.vector.tensor_tensor(
                        out=acc[:, seg, :],
                        in0=blc_sb[:, seg].to_broadcast([128, gs, G]),
                        in1=_midbcast(io64[:], gs),
                        op=AL.is_equal)
                nc.vector.tensor_tensor(
                    out=acc[:, seg, :], in0=acc[:, seg, :],
                    in1=selfw_sb[:, seg].to_broadcast([128, gs, G]),
                    op=AL.mult)

            prev_ag_out = [None]

            def ag_quarter(pk, qo):
                nc.gpsimd.collective_compute(
                    "AllGather", AL.bypass,
                    replica_groups=[list(range(cfg.n_cores))],
                    ins=[shardq[qo][:]],
                    outs=[uq[pk % 2][qo][:]])
                prev_ag_out[0] = uq[pk % 2][qo]

            def write_quarter(pk, qo):
                if QUARTER_AG:
                    # serialize collectives: a 1-row token copy from the
                    # previous AG's output into this quarter (overwritten
                    # below) makes this quarter's AG wait for the previous
                    # collective's completion.
                    if prev_ag_out[0] is not None:
                        nc.sync.dma_start(out=shardq[qo][0:1, :],
                                          in_=prev_ag_out[0][0:1, :])
                    nc.sync.dma_start(
                        out=shardq[qo][:].rearrange("(b p) f -> p b f", p=128),
                        in_=acc[:, qo * BPQ:(qo + 1) * BPQ, :])
                    if pk < LAYERS - 1:
                        ag_quarter(pk, qo)
                else:
                    shard_pbf = shard[:].rearrange("(b p) f -> p b f", p=128)
                    nc.sync.dma_start(
                        out=shard_pbf[:, qo * BPQ:(qo + 1) * BPQ, :],
                        in_=acc[:, qo * BPQ:(qo + 1) * BPQ, :])

            def end_pass_ags(pk):
                if pk < LAYERS - 1 and not QUARTER_AG:
                    nc.gpsimd.collective_compute(
                        "AllGather", AL.bypass,
                        replica_groups=[list(range(cfg.n_cores))],
                        ins=[shard[:]], outs=[ufull[pk % 2][:]])

            # ---------------- pass 0: quarter-major (messages from HBM)
            for qo in range(NW):
                for g in (2 * qo, 2 * qo + 1):
                    self_init(0, g)
                    for w in range(NW):
                        group_window(0, g, w)
                write_quarter(0, qo)
            end_pass_ags(0)

            # ---------------- passes 1..L-1: window-major
            for pk in range(1, LAYERS):
                for w in range(NW):
                    for g in range(2 * NW):
                        if w == 0:
                            self_init(pk, g)
                        group_window(pk, g, w)
                        if w == NW - 1 and g % 2 == 1 and pk < LAYERS - 1:
                            write_quarter(pk, (g - 1) // 2)
                end_pass_ags(pk)

            # final dense matmul: out_part[g, f] = sum_n u3[n, g] * Xp[n, f]
            fps = ep_pool.tile([G, F], f32, tag="ep")
            xp_pbf = Xp[:].rearrange("(b p) f -> p b f", p=128)
            for qo in range(NW):
                xpt = xp_pool.tile([128, BPQ, F], bf16, tag="xp")
                nc.sync.dma_start(
                    out=xpt[:, :, :],
                    in_=xp_pbf[:, qo * BPQ:(qo + 1) * BPQ, :])
                for b in range(BPQ):
                    gb = qo * BPQ + b
                    nc.tensor.matmul(fps[:], lhsT=acc[:, gb, :],
                                     rhs=xpt[:, b, :],
                                     start=(gb == 0), stop=(gb == NBLK - 1))
            outp = fin_pool.tile([G, F], f32, tag="outp")
            nc.vector.tensor_copy(out=outp[:], in_=fps[:])
            nc.sync.dma_start(out=arin[:], in_=outp[:])
            nc.gpsimd.collective_compute(
                "AllReduce", AL.add,
                replica_groups=[list(range(cfg.n_cores))],
                ins=[arin[:]], outs=[arout[:]])
            ar_sb = fin_pool.tile([G, F], f32, tag="arsb")
            nc.sync.dma_start(out=ar_sb[:], in_=arout[:])

            # epilogue: res^T ; out = (res^T W012) / counts
            tps = ep_pool.tile([128, G], f32, tag="ep")
            nc.tensor.transpose(out=tps[:], in_=ar_sb[:, :],
                                identity=ident_sb[:G, :G])
            resT = fin_pool.tile([128, G], f32, tag="resT")
            nc.vector.tensor_copy(out=resT[:], in_=tps[:])
            ops = ep_pool.tile([G, F], f32, tag="ep")
            nc.tensor.matmul(ops[:], lhsT=resT[:], rhs=w012[:], start=True,
                             stop=True)
            icnt = fin_pool.tile([G, 1], f32, tag="icnt")
            nc.sync.dma_start(out=icnt[:], in_=inv_cnt[:])
            fin = fin_pool.tile([G, F], f32, tag="finout")
            nc.vector.tensor_scalar_mul(fin[:], ops[:], icnt[:])
            nc.sync.dma_start(out=out_ext[:], in_=fin[:])

    nc.compile()
    return nc


def make_in_maps(cfg, aux):
    in_maps = []
    for c in range(cfg.n_cores):
        in_maps.append({
            "gidx16": np.ascontiguousarray(aux["gidx16"][c]),
            "nrm_sm": np.ascontiguousarray(aux["nrm_sm"][c]),
            "doff_sm": np.ascontiguousarray(aux["doff_sm"][c]),
            "oh_hbm": np.ascontiguousarray(aux["oh_hbm"][c]),
            "msg0_hbm": np.ascontiguousarray(aux["msg0_hbm"][c]),
            "selfw": np.ascontiguousarray(aux["selfw"][c]),
            "batchloc": np.ascontiguousarray(aux["batchloc"][c]),
            "Xpb": np.ascontiguousarray(aux["Xpb"][c]),
            "inv_cnt": aux["inv_cnt"],
            "W0T": aux["W0T"], "W1T": aux["W1T"], "W2": aux["W2"],
        })
    return in_maps


_PROGRAM_CACHE = {}


def kernel(**inputs):
    from concourse.bass_utils import run_bass_kernel_spmd

    cfg = FULL_CFG
    x = np.asarray(inputs["x"], dtype=np.float32)
    edge_index = np.asarray(inputs["edge_index"])
    edge_attr = np.asarray(inputs["edge_attr"], dtype=np.float32)
    batch = np.asarray(inputs["batch"])
    Ws = np.asarray(inputs["Ws"], dtype=np.float32)
    bs = np.asarray(inputs["bs"], dtype=np.float32)
    assert not np.any(bs), "nonzero biases not supported by this kernel build"

    aux = host_prep(cfg, x, edge_index, edge_attr, batch, Ws, bs)
    lay = aux["layout"]
    key = ("v3", lay.key(), USE_PREP, QUARTER_AG, PARITY_MM)
    if key not in _PROGRAM_CACHE:
        _PROGRAM_CACHE[key] = build_program(cfg, lay)
    nc = _PROGRAM_CACHE[key]
    in_maps = make_in_maps(cfg, aux)
    res = run_bass_kernel_spmd(nc, in_maps, core_ids=list(range(cfg.n_cores)))
    return np.asarray(res.results[0]["out"], dtype=np.float32)
